# revision 52
# baseline (speedup 1.0000x reference)
"""Trainium2 Bass kernel for nn_CrossAttention (B=8, L=1024, QD=1024, KVD=768, H=16).

Sharding: data-parallel over batch across the 8 NeuronCores (1 batch row each).

Host-side marshalling (inside kernel()): q/k/v and the projection weights
are pre-transposed into "contraction-dim-on-partitions" tile layouts and
pre-cast (fp8e4m3 for the projection operands, bf16 for Wo), so the device
does zero transposes and zero dtype conversion on the critical path.  A
separate fp32 copy of q feeds the residual at the tail.

Per-core pipeline:
  - B1/B2/B3 projections run as fp8e4m3 DoubleRow matmuls (256-deep
    virtual contraction -> half the matmul count), accumulating fp32 in
    PSUM, evicted to bf16 qhT/khT/vh via DVE (+bias).  Final L2 error vs
    the fp32 reference is ~6e-4 (gate 2e-2) because the attention output
    is small relative to the fp32 residual + layernorm path.
  - scores: bf16, two heads row-packed on the PE (base partitions 0/64);
    exp folds mask+scale via the ACT affine path.  The ACT exp stream is
    the attention metronome; B and attnv work drain through a paced
    backlog between per-mt score bursts so the PE never head-of-line
    blocks on PSUM while ACT grinds.
  - attnV: bf16 with the [ones|vh] stationary trick (PSUM rows 0:64 =
    softmax denominator); fast approx reciprocal + multiply on DVE.
    attnv for pair p-1 drains interleaved under pair p's exp stream.
    (Note: an fp8-DoubleRow attnV variant was tried and is ~10us faster
    but intermittently produced NaNs on hardware despite all of its
    pieces passing isolated micro-tests; kept bf16 for correctness.)
  - out-projection (bf16) accumulates oT @ WoT per l-tile; bo is folded
    into the prefetched fp32 q rows on DVE, then residual add + layernorm
    (stats on DVE, rsqrt on ACT, normalize on DVE).
"""

import numpy as np
import ml_dtypes
from collections import deque

import concourse.bass as bass
import concourse.mybir as mybir
import concourse.tile as tile
from concourse import bacc
from concourse.bass_utils import run_bass_kernel_spmd

F32 = mybir.dt.float32
BF16 = mybir.dt.bfloat16
F8 = mybir.dt.float8e4
U8 = mybir.dt.uint8
DR = mybir.MatmulPerfMode.DoubleRow

B = 8
L = 1024
C = 1024      # QD
KV = 768      # KVD
H = 16
DH = 64
P = 128
LT = L // P          # 8 l-tiles
CT = C // P          # 8 contraction tiles (model dim)
KT = KV // P         # 6 contraction tiles (kv dim)
DT = C // P          # 8 d-tiles
NH = C // 512        # 2 free-dim halves (N=512 per PSUM bank)
SCALE = DH ** -0.5
EPS = 1e-5
MASK_NEG = -30000.0
LN16 = float(np.log(16.0))   # exp outputs 16*p so small p stay normal in fp8

Exp = mybir.ActivationFunctionType.Exp
Sqrt = mybir.ActivationFunctionType.Sqrt
Identity = mybir.ActivationFunctionType.Identity
MULT = mybir.AluOpType.mult
ADD = mybir.AluOpType.add

TRACE = False
LAST_RESULT = None
_CACHE = {}

PT_BUFS = 16          # live exp-output tiles ([128,2,1024] fp8, 2KB/partition)

# emission-time cost estimates (ns) used to pace the backlog drain
MM_NS = 225
EXP_NS = 1300
B1_NS = 4 * MM_NS + 650
B2_NS = 3 * MM_NS + 650
B3_NS = 3 * MM_NS + 700
AV_NS = 16 * MM_NS + 1200


def _bcast_ap(handle, parts):
    apx = handle[:]
    return bass.AP(tensor=apx.tensor, offset=apx.offset,
                   ap=[[0, parts]] + [list(x) for x in apx.ap])


def build(apply_gb=False):
    nc = bacc.Bacc("TRN2", target_bir_lowering=False)

    qT_in = nc.dram_tensor("qT", [P, CT, L], F8, kind="ExternalInput")
    kT_in = nc.dram_tensor("kT", [P, KT, L], F8, kind="ExternalInput")
    vT_in = nc.dram_tensor("vT", [P, KT, L], F8, kind="ExternalInput")
    WqT_in = nc.dram_tensor("WqT", [P, CT, C], F8, kind="ExternalInput")
    WkT_in = nc.dram_tensor("WkT", [P, KT, C], F8, kind="ExternalInput")
    WvT_in = nc.dram_tensor("WvT", [P, KT, C], F8, kind="ExternalInput")
    WoT_in = nc.dram_tensor("WoT", [P, DT, C], BF16, kind="ExternalInput")
    q_in = nc.dram_tensor("q", [L, C], F32, kind="ExternalInput")
    mr_in = nc.dram_tensor("maskr", [P, LT], U8, kind="ExternalInput")
    bqr_in = nc.dram_tensor("bqr", [P, DT], F32, kind="ExternalInput")
    bkr_in = nc.dram_tensor("bkr", [P, DT], F32, kind="ExternalInput")
    bv_in = nc.dram_tensor("bv", [C], F32, kind="ExternalInput")
    bo_in = nc.dram_tensor("bo", [C], F32, kind="ExternalInput")
    gamma_in = nc.dram_tensor("gamma", [C], F32, kind="ExternalInput")
    beta_in = nc.dram_tensor("beta", [C], F32, kind="ExternalInput")
    y_out = nc.dram_tensor("y", [L, C], F32, kind="ExternalOutput")

    with tile.TileContext(nc) as tc:
        with (
            tc.tile_pool(name="cst", bufs=1) as cst,
            tc.tile_pool(name="persist", bufs=1) as persist,
            tc.tile_pool(name="psum", bufs=1, space="PSUM") as psum,
        ):
            # ---- persistent intermediates
            qhT = persist.tile([P, DT, L], BF16)          # d on partitions
            khT = persist.tile([P, DT, L], BF16)
            vh_aug = persist.tile([P, LT, H * P], BF16)   # per m-tile: 16x[64 ones | 64 vh]
            oT = persist.tile([P, DT, L], BF16)

            # HAM warm-up: PE busy from t~1us so B1/B2 run at 2.4GHz
            warm = cst.tile([P, 512], BF16)
            nc.vector.memset(warm, 0.0)
            wps = psum.tile([P, 512], F32, tag="b", bufs=1, name="warmps")
            for _ in range(20):
                nc.tensor.matmul(wps, warm[:, 0:P], warm, start=True,
                                 stop=True)

            eps_sb = cst.tile([P, 1], F32)
            nc.vector.memset(eps_sb, EPS)
            nc.vector.memset(vh_aug[:], 1.0)
            mask_u8 = cst.tile([P, LT], U8)
            bq_sb = cst.tile([P, DT], F32)
            bk_sb = cst.tile([P, DT], F32)
            mask_bias = cst.tile([P, LT], F32)
            bv_b = cst.tile([P, C], BF16)     # broadcast biases (free-dim)
            bo_b = cst.tile([P, C], BF16)
            if apply_gb:
                gamma_b = cst.tile([P, C], F32)
                beta_b = cst.tile([P, C], F32)
            else:
                gamma_b = beta_b = None

            with tc.tile_pool(name="stIn", bufs=1) as stIn:
                qT = stIn.tile([P, CT, L], F8)
                WqT = stIn.tile([P, CT, C], F8)
                kT = stIn.tile([P, KT, L], F8)
                WkT = stIn.tile([P, KT, C], F8)
                WoT = stIn.tile([P, DT, C], BF16)

                # gate loads first, alternating queues for issue overlap;
                # order = need order for B1(0)/B2(0)/scores(0).  The tiny
                # consts are host-rearranged so they load contiguously.
                nc.sync.dma_start(bq_sb, bqr_in[:])
                nc.sync.dma_start(bk_sb, bkr_in[:])
                nc.sync.dma_start(mask_u8, mr_in[:])
                nc.vector.tensor_copy(mask_bias, mask_u8)
                nc.vector.tensor_scalar(mask_bias, mask_bias, -MASK_NEG,
                                        MASK_NEG, MULT, ADD)
                nc.sync.dma_start(qT[:, :, 0:512], qT_in[:, :, 0:512])
                nc.gpsimd.dma_start(WqT, WqT_in[:])
                nc.sync.dma_start(qT[:, :, 512:1024], qT_in[:, :, 512:1024])
                nc.gpsimd.dma_start(kT[:, :, 0:512], kT_in[:, :, 0:512])
                nc.sync.dma_start(WkT, WkT_in[:])
                nc.gpsimd.dma_start(kT[:, :, 512:1024], kT_in[:, :, 512:1024])

                # -------- compute building blocks (fp8 DoubleRow projections)
                def b1_group(dt, lh):
                    ps = psum.tile([P, 512], F32, tag="b", bufs=1,
                                   name=f"psb1_{dt}_{lh}")
                    for c2 in range(CT // 2):
                        nc.tensor.matmul(
                            ps, WqT[:, 2 * c2:2 * c2 + 2, dt * P:(dt + 1) * P],
                            qT[:, 2 * c2:2 * c2 + 2, lh * 512:(lh + 1) * 512],
                            start=(c2 == 0), stop=(c2 == CT // 2 - 1),
                            perf_mode=DR)
                    nc.vector.tensor_scalar_add(
                        qhT[:, dt, lh * 512:(lh + 1) * 512], ps,
                        bq_sb[:, dt:dt + 1])

                def b2_group(dt, lh):
                    ps = psum.tile([P, 512], F32, tag="b", bufs=1,
                                   name=f"psb2_{dt}_{lh}")
                    for c2 in range(KT // 2):
                        nc.tensor.matmul(
                            ps, WkT[:, 2 * c2:2 * c2 + 2, dt * P:(dt + 1) * P],
                            kT[:, 2 * c2:2 * c2 + 2, lh * 512:(lh + 1) * 512],
                            start=(c2 == 0), stop=(c2 == KT // 2 - 1),
                            perf_mode=DR)
                    nc.vector.tensor_scalar_add(
                        khT[:, dt, lh * 512:(lh + 1) * 512], ps,
                        bk_sb[:, dt:dt + 1])

                vstage = {}

                def b3_group(mt, dh2):
                    vT, WvT = vstage["vT"], vstage["WvT"]
                    ps = psum.tile([P, 512], F32, tag="b", bufs=1,
                                   name=f"psb3_{mt}_{dh2}")
                    for c2 in range(KT // 2):
                        nc.tensor.matmul(
                            ps, vT[:, 2 * c2:2 * c2 + 2, mt * P:(mt + 1) * P],
                            WvT[:, 2 * c2:2 * c2 + 2, dh2 * 512:(dh2 + 1) * 512],
                            start=(c2 == 0), stop=(c2 == KT // 2 - 1),
                            perf_mode=DR)
                    dst = vh_aug[:, mt, :].rearrange("p (h x) -> p h x", x=P)
                    dst = dst[:, dh2 * 8:(dh2 + 1) * 8, DH:P]
                    nc.vector.tensor_add(
                        dst, ps[:].rearrange("p (h d) -> p h d", d=DH),
                        bv_b[:, dh2 * 512:(dh2 + 1) * 512].rearrange(
                            "p (h d) -> p h d", d=DH))

                pts = {}

                with (
                    tc.tile_pool(name="ptp", bufs=PT_BUFS) as ptp,
                    tc.tile_pool(name="recp", bufs=1) as recp,
                ):
                    def scores_mt(pair, mt):
                        sc = []
                        for hh in range(2):
                            s = psum.tile([P, L], F32, tag="sc", bufs=2,
                                          name=f"sc{pair}_{mt}_{hh}")
                            sc.append(s)
                            p0 = hh * DH
                            for lh in range(NH):
                                nc.tensor.matmul(
                                    s[:, lh * 512:(lh + 1) * 512],
                                    khT[p0:p0 + DH, pair, mt * P:(mt + 1) * P],
                                    qhT[p0:p0 + DH, pair, lh * 512:(lh + 1) * 512],
                                    start=True, stop=True)
                        for hh in range(2):
                            pt = ptp.tile([P, L], BF16, tag="pt",
                                          name=f"pt{pair}_{mt}_{hh}")
                            pts[(pair, mt, hh)] = pt
                            nc.scalar.activation(pt, sc[hh], Exp,
                                                 bias=mask_bias[:, mt:mt + 1],
                                                 scale=SCALE)

                    def attnv_half(pair, hh):
                        h = 2 * pair + hh
                        avs = [psum.tile([P, 512], F32, tag="av", bufs=3,
                                         name=f"av{pair}_{hh}_{lh}")
                               for lh in range(NH)]
                        for mt in range(LT):
                            for lh in range(NH):
                                nc.tensor.matmul(
                                    avs[lh],
                                    vh_aug[:, mt, h * P:(h + 1) * P],
                                    pts[(pair, mt, hh)][:, lh * 512:(lh + 1) * 512],
                                    start=(mt == 0), stop=(mt == LT - 1))
                        for lh in range(NH):
                            av = avs[lh]
                            rec = recp.tile([P, 512], F32,
                                            name=f"rec{pair}_{hh}_{lh}",
                                            tag="rec")
                            nc.vector.reciprocal_approx_fast(rec[0:DH, :],
                                                             av[0:DH, :])
                            nc.vector.tensor_mul(
                                oT[hh * DH:(hh + 1) * DH, pair,
                                   lh * 512:(lh + 1) * 512],
                                av[DH:P, :], rec[0:DH, :])
                        for mt in range(LT):
                            del pts[(pair, mt, hh)]

                    # ---- backlog machinery: pace PE emission against ACT
                    avq = deque()
                    backlog = deque()
                    state = {"pe": 0.0, "act": 0.0}

                    def emit(it):
                        kind = it[0]
                        if kind == "b1":
                            b1_group(it[1], it[2]); state["pe"] += B1_NS
                        elif kind == "b2":
                            b2_group(it[1], it[2]); state["pe"] += B2_NS
                        elif kind == "b3":
                            b3_group(it[1], it[2]); state["pe"] += B3_NS
                        else:
                            attnv_half(it[1], it[2]); state["pe"] += AV_NS

                    def pull(pred):
                        keep = deque()
                        while backlog:
                            it = backlog.popleft()
                            if pred(it):
                                emit(it)
                            else:
                                keep.append(it)
                        backlog.extend(keep)

                    def drain(budget_ns):
                        while (avq or backlog) and budget_ns > 0:
                            it = avq.popleft() if avq else backlog.popleft()
                            emit(it)
                            budget_ns -= AV_NS if it[0] == "av" else \
                                {"b1": B1_NS, "b2": B2_NS, "b3": B3_NS}[it[0]]

                    def do_scores(pair):
                        pull(lambda it: it[0] in ("b1", "b2") and it[1] == pair)
                        for mt in range(LT):
                            scores_mt(pair, mt)
                            state["act"] += 2 * EXP_NS
                            state["pe"] += 4 * MM_NS
                            drain(state["act"] - state["pe"])

                    # ---- slot 0: pair-0 projections emitted directly
                    b1_group(0, 0); b2_group(0, 0)
                    b1_group(0, 1); b2_group(0, 1)
                    state["pe"] += 2 * B1_NS + 2 * B2_NS

                    # backlog: B3 in front (gates attnv(0)), then B1/B2
                    for dh2 in range(NH):
                        for mt in range(LT):
                            backlog.append(("b3", mt, dh2))
                    for dt in range(1, DT):
                        backlog.append(("b1", dt, 0))
                        backlog.append(("b1", dt, 1))
                        backlog.append(("b2", dt, 0))
                        backlog.append(("b2", dt, 1))

                    with tc.tile_pool(name="stV", bufs=1) as stV:
                        vT = stV.tile([P, KT, L], F8)
                        WvT = stV.tile([P, KT, C], F8)
                        vstage["vT"], vstage["WvT"] = vT, WvT
                        nc.sync.dma_start(vT, vT_in[:])
                        nc.gpsimd.dma_start(WvT, WvT_in[:])
                        nc.sync.dma_start(WoT, WoT_in[:])
                        nc.gpsimd.dma_start(bv_b, _bcast_ap(bv_in, P))
                        nc.gpsimd.dma_start(bo_b, _bcast_ap(bo_in, P))
                        if apply_gb:
                            nc.gpsimd.dma_start(gamma_b, _bcast_ap(gamma_in, P))
                            nc.gpsimd.dma_start(beta_b, _bcast_ap(beta_in, P))

                        do_scores(0)
                        do_scores(1)
                        pull(lambda it: it[0] == "b3")   # vh complete
                        avq.append(("av", 0, 0))
                        avq.append(("av", 0, 1))
                        do_scores(2)

                    for pair in range(3, H // 2):
                        avq.append(("av", pair - 2, 0))
                        avq.append(("av", pair - 2, 1))
                        if pair == H // 2 - 1:
                            avq.append(("av", pair - 1, 0))
                            avq.append(("av", pair - 1, 1))
                        do_scores(pair)
                    avq.append(("av", H // 2 - 1, 0))
                    avq.append(("av", H // 2 - 1, 1))
                    pull(lambda it: True)
                    while avq:
                        emit(avq.popleft())

                # ------------- out-projection + residual + layernorm
                with (
                    tc.tile_pool(name="qrp", bufs=1) as qrp,
                    tc.tile_pool(name="dwork", bufs=2) as dwork,
                    tc.tile_pool(name="dsmall", bufs=8) as dsmall,
                ):
                    # rotating prefetch (4 deep) of q rows for the residual
                    qrs = {}

                    def load_qr(lt):
                        qr = qrp.tile([P, C], F32, name=f"qr{lt}", tag="qr",
                                      bufs=4)
                        nc.sync.dma_start(qr, q_in[lt * P:(lt + 1) * P, :])
                        nc.vector.tensor_add(qr, qr, bo_b)
                        qrs[lt] = qr

                    for lt in range(4):
                        load_qr(lt)
                    for lt in range(LT):
                        if lt + 4 < LT:
                            load_qr(lt + 4)
                        yp = psum.tile([P, C], F32, tag="sc", bufs=2,
                                       name=f"yp{lt}")
                        for dt in range(DT):
                            for ch2 in range(NH):
                                nc.tensor.matmul(
                                    yp[:, ch2 * 512:(ch2 + 1) * 512],
                                    oT[:, dt, lt * P:(lt + 1) * P],
                                    WoT[:, dt, ch2 * 512:(ch2 + 1) * 512],
                                    start=(dt == 0), stop=(dt == DT - 1))
                        ysb = dwork.tile([P, C], F32, tag="ysb")
                        nc.vector.tensor_add(ysb, yp, qrs[lt])
                        st = dsmall.tile([P, 2, 6], F32, tag="st")
                        nc.vector.bn_stats(st[:, 0, :], ysb[:, 0:512])
                        nc.vector.bn_stats(st[:, 1, :], ysb[:, 512:1024])
                        mv = dsmall.tile([P, 2], F32, tag="mv")
                        nc.vector.bn_aggr(mv, st)
                        rstd = dsmall.tile([P, 1], F32, tag="rstd")
                        nc.scalar.activation(rstd, mv[:, 1:2], Sqrt,
                                             bias=eps_sb[:, 0:1])
                        nc.vector.reciprocal(rstd, rstd)
                        nmr = dsmall.tile([P, 1], F32, tag="nmr")
                        nc.vector.tensor_mul(nmr, mv[:, 0:1], rstd)
                        nc.vector.tensor_scalar_mul(nmr, nmr, -1.0)
                        yn = dwork.tile([P, C], F32, tag="yn")
                        # normalize on GpSimd (idle at the tail; DVE paces
                        # the out-projection otherwise)
                        nc.gpsimd.tensor_scalar(yn, ysb, rstd[:, 0:1],
                                                nmr[:, 0:1], MULT, ADD)
                        if apply_gb:
                            nc.vector.tensor_mul(yn, yn, gamma_b)
                            nc.gpsimd.tensor_add(yn, yn, beta_b)
                        nc.sync.dma_start(y_out[lt * P:(lt + 1) * P, :], yn)

    nc.compile()
    return nc


def _get_nc(apply_gb):
    key = ("nc", apply_gb)
    if key not in _CACHE:
        _CACHE[key] = build(apply_gb)
    return _CACHE[key]


BF = ml_dtypes.bfloat16
E4 = ml_dtypes.float8_e4m3


def _t_tiles(x, dt=E4):
    """[R, C] -> transposed tile layout [128, C//128, R], cast, contiguous."""
    r, c = x.shape
    return np.ascontiguousarray(
        x.reshape(r, c // P, P).transpose(2, 1, 0).astype(dt))


def kernel(**inputs) -> np.ndarray:
    global LAST_RESULT
    gamma = np.asarray(inputs["gamma"], dtype=np.float32)
    beta = np.asarray(inputs["beta"], dtype=np.float32)
    apply_gb = not (np.all(gamma == 1.0) and np.all(beta == 0.0))
    nc = _get_nc(apply_gb)
    q = np.ascontiguousarray(np.asarray(inputs["q"], dtype=np.float32))
    k = np.asarray(inputs["k"], dtype=np.float32)
    v = np.asarray(inputs["v"], dtype=np.float32)
    mask = np.ascontiguousarray(np.asarray(inputs["key_padding_mask"]).astype(np.uint8))
    WqT = _t_tiles(np.asarray(inputs["Wq"], dtype=np.float32))
    WkT = _t_tiles(np.asarray(inputs["Wk"], dtype=np.float32))
    WvT = _t_tiles(np.asarray(inputs["Wv"], dtype=np.float32))
    WoT = _t_tiles(np.asarray(inputs["Wo"], dtype=np.float32), BF)
    shared = {"WqT": WqT, "WkT": WkT, "WvT": WvT, "WoT": WoT}
    for name in ("bv", "bo", "gamma", "beta"):
        shared[name] = np.ascontiguousarray(
            np.asarray(inputs[name], dtype=np.float32))
    for name in ("bq", "bk"):
        vec = np.asarray(inputs[name], dtype=np.float32)
        shared[name + "r"] = np.ascontiguousarray(vec.reshape(DT, P).T)
    in_maps = []
    for b in range(B):
        m = {"qT": _t_tiles(q[b]), "kT": _t_tiles(k[b]), "vT": _t_tiles(v[b]),
             "q": q[b],
             "maskr": np.ascontiguousarray(mask[b].reshape(LT, P).T)}
        m.update(shared)
        in_maps.append(m)
    LAST_RESULT = run_bass_kernel_spmd(nc, in_maps, core_ids=list(range(B)), trace=TRACE)
    return np.stack([r["y"] for r in LAST_RESULT.results], axis=0)
